# revision 18
# baseline (speedup 1.0000x reference)
"""Trainium2 Bass kernel: single-head attention with RoPE and the reference's
multiplicative causal mask (masked logits stay 0 -> exp(0)=1, so masked
positions contribute exp(0)=1 to softmax -- attention is dense over the
upper triangle too, but those probabilities are a constant 1/Z).

Sharding: 8 cores = 4 batches x 2 row-parity halves. Core (b, h) owns the
interleaved rows x[b, h::2] -- with this split the causal-mask tile classes
are identical on every core, so fully-masked S^T tiles are skipped
statically (same SPMD graph everywhere) and their P==1 contribution enters
as a per-dout constant (onesum) plus a denominator offset.

Per core: project K (dlow-outer so each weight panel loads once), RoPE
on-chip, single AllGather of roped K within the 2-core pair; V projection
streamed by dout block (wv loaded per block, not upfront) with the gather
split in two dout halves; Q projection reusing the cos/sin tables kept in
SBUF since the K phase; then S^T = K@Q^T, P = exp(mask*S^T/sqrt(S)) with
the onesum chains interleaved to keep the PE clock-gate warm, and
O^T = V^T@P^T / denom emitted dout-major so outputs stream back in 512KB
DMAs. Output is O^T per core; the host transposes and reassembles.
"""

import sys

for _p in ("/opt/trn_rl_repo", "/root/.axon_site/_ro/trn_rl_repo"):
    if _p not in sys.path:
        sys.path.append(_p)

import math

import ml_dtypes
import numpy as np

BF16 = ml_dtypes.bfloat16

B, S, D = 4, 2048, 2048
NOWN = 1024  # query rows per core
P = 128  # partitions
KD = D // P  # 16 feature chunks
NCJ = S // P  # 16 key chunks
N_CORES = 8
PAIRS = [[0, 1], [2, 3], [4, 5], [6, 7]]
FB = 512  # matmul moving free-dim block
NB = NOWN // FB  # 2 blocks of own rows
SCALE = 1.0 / math.sqrt(S)  # reference scales by sqrt(seq_len), not sqrt(D)

# Quarter-granularity mask staircase (identical on every core with
# interleaved rows): for i-quarter q (256 columns) and j-chunk jc with
# m = (jc % 8) // 2:  q < m -> fully masked (skipped, P==1);
# q == m -> mixed (mask applied); q > m -> fully unmasked.
QW = 256
NQ = NOWN // QW  # 4 quarters


def _m_of(jc):
    return (jc % 8) // 2


# chunks contributing computed S tiles for quarter q
def _comp(q):
    return [jc for jc in range(NCJ) if _m_of(jc) <= q]


# onesum stages: chunks that become skipped when stepping down a quarter
OS_STAGES = [
    [jc for jc in range(NCJ) if _m_of(jc) == 3],  # skipped for q<=2
    [jc for jc in range(NCJ) if _m_of(jc) == 2],  # additionally for q<=1
    [jc for jc in range(NCJ) if _m_of(jc) == 1],  # additionally for q==0
]

_CACHE = {}


def _build():
    import concourse.bass as bass  # noqa: F401
    import concourse.tile as tile
    from concourse import bacc, mybir

    f32 = mybir.dt.float32
    bf16 = mybir.dt.bfloat16

    nc = bacc.Bacc(
        "TRN2", target_bir_lowering=False, debug=False, num_devices=N_CORES
    )

    x_ext = nc.dram_tensor("x_t", [P, KD, NOWN], bf16, kind="ExternalInput").ap()
    wq_ext = nc.dram_tensor("wq", [KD, P, KD, P], bf16, kind="ExternalInput").ap()
    wk_ext = nc.dram_tensor("wk", [KD, P, KD, P], bf16, kind="ExternalInput").ap()
    # wv re-laid host-side as [wb, p, k, 512] so each dout block is contiguous
    wv_ext = nc.dram_tensor("wv", [4, P, KD, FB], bf16, kind="ExternalInput").ap()
    cos_ext = nc.dram_tensor("cos_t", [KD, P, NOWN], bf16, kind="ExternalInput").ap()
    sin_ext = nc.dram_tensor("sin_t", [KD, P, NOWN], bf16, kind="ExternalInput").ap()
    mask_ext = nc.dram_tensor("mask_t", [NCJ, P, QW], bf16, kind="ExternalInput").ap()
    out_ext = nc.dram_tensor("out", [D, NOWN], f32, kind="ExternalOutput").ap()

    with tile.TileContext(nc) as tc:
        with (
            tc.tile_pool(name="dram", bufs=1, space="DRAM") as dram,
            tc.tile_pool(name="psum", bufs=5, space="PSUM") as psum,
            tc.tile_pool(name="dnsum", bufs=1, space="PSUM") as dnsum,
            tc.tile_pool(name="persist", bufs=1) as persist,
            tc.tile_pool(name="tmp", bufs=6) as tmp,
            tc.tile_pool(name="strm", bufs=8) as strm,
        ):
            kt_local = dram.tile([KD, NB, P, FB], bf16)  # [dc, nb, p, row] contiguous
            v_local = dram.tile([2, NCJ // 2, P, NOWN], bf16)  # dout halves
            kt_gath_a = dram.tile([2, KD // 2, NB, P, FB], bf16)
            kt_gath_b = dram.tile([2, KD // 2, NB, P, FB], bf16)
            v_gath = dram.tile([2, 2, NCJ // 2, P, NOWN], bf16)

            def kt_g(jc, half):
                # [P, KD/2, 128] slab view: transposed [KD/2, P, 128] dram slice
                r, nb, c0 = jc // 8, (jc % 8) // 4, ((jc % 8) % 4) * P
                g = kt_gath_a if half == 0 else kt_gath_b
                return g[r, :, nb, :, c0 : c0 + P].transpose([1, 0, 2])

            ones_col = persist.tile([P, 1], bf16)
            nc.vector.memset(ones_col, 1.0)
            ones_row = persist.tile([1, P], f32)
            nc.vector.memset(ones_row, 1.0)

            qt_sb = persist.tile([P, KD, NOWN], bf16)

            # warm-up matmuls on a memset tile: ~5us of PE activity during the
            # initial DMA window flips the HAM clock-gate to 2.4GHz before the
            # first real chain issues (otherwise the first ~3.4us run at half
            # clock), at zero cost since the PE would be idle anyway
            wu_sb = persist.tile([P, FB], bf16, name="wu_sb")
            nc.vector.memset(wu_sb, 0.0)
            for i in range(12):
                ps_w = psum.tile([P, FB], f32, tag="ps", name=f"wup{i}")
                nc.tensor.matmul(
                    ps_w, lhsT=wu_sb[:, :P], rhs=wu_sb, start=True, stop=True
                )

            # ---- staged loads: weights dlow=0 first, then x, then cos/sin ----
            cs_pool = tc.alloc_tile_pool(name="cs_pool", bufs=1)
            cos_sb = cs_pool.tile([P, KD, NOWN], bf16, name="cos_sb")
            sin_sb = cs_pool.tile([P, KD, NOWN], bf16, name="sin_sb")

            # x in 8 independent tiles so the chunk DMAs run in parallel
            x_pool = tc.alloc_tile_pool(name="x_pool", bufs=1)
            x_ts = [
                x_pool.tile([P, 2, NOWN], bf16, name=f"x_sb{i}") for i in range(8)
            ]
            x_dma_engines = [
                nc.sync,
                nc.scalar,
                nc.sync,
                nc.scalar,
                nc.sync,
                nc.scalar,
                nc.sync,
                nc.scalar,
            ]

            def x_ref(k):
                return x_ts[k // 2][:, k % 2, :]

            _panel_engines = [nc.sync, nc.scalar]

            def load_panels(wpool, w_ext, dlow):
                dhigh = dlow + KD // 2
                e0 = _panel_engines[dlow % 2]
                e1 = _panel_engines[(dlow + 1) % 2]
                w_lo = wpool.tile([P, KD, P], bf16, tag="wp", name=f"wlo{dlow}")
                e0.dma_start(out=w_lo, in_=w_ext[dlow])
                w_hi = wpool.tile([P, KD, P], bf16, tag="wp", name=f"whi{dlow}")
                e1.dma_start(out=w_hi, in_=w_ext[dhigh])
                return w_lo, w_hi

            def rope_pair(panels, dlow, nb, out_ap, post):
                """One (dlow, nb) unit: two projection chains + rope."""
                dhigh = dlow + KD // 2
                sl = slice(nb * FB, (nb + 1) * FB)
                cos_t = cos_sb[:, dlow, sl]
                sin_t = sin_sb[:, dlow, sl]
                cos_h = cos_sb[:, dhigh, sl]
                sin_h = sin_sb[:, dhigh, sl]
                w_lo, w_hi = panels
                ps_lo = psum.tile([P, FB], f32, tag="ps", name=f"plo{dlow}{nb}")
                for k in range(KD):
                    nc.tensor.matmul(
                        ps_lo,
                        lhsT=w_lo[:, k, :],
                        rhs=x_ref(k)[:, sl],
                        start=(k == 0),
                        stop=(k == KD - 1),
                    )
                ps_hi = psum.tile([P, FB], f32, tag="ps", name=f"phi{dlow}{nb}")
                for k in range(KD):
                    nc.tensor.matmul(
                        ps_hi,
                        lhsT=w_hi[:, k, :],
                        rhs=x_ref(k)[:, sl],
                        start=(k == 0),
                        stop=(k == KD - 1),
                    )
                # rope low half: out = lo*cos_l - hi*sin_l
                t1 = tmp.tile([P, FB], f32, tag="t", name=f"t1{dlow}{nb}")
                nc.vector.tensor_mul(t1, ps_lo, cos_t)
                t2 = tmp.tile([P, FB], f32, tag="t", name=f"t2{dlow}{nb}")
                nc.vector.tensor_mul(t2, ps_hi, sin_t)
                o_lo = out_ap(dlow, nb)
                nc.vector.tensor_sub(o_lo, t1, t2)
                if post is not None:
                    post(dlow, nb, o_lo)
                # rope high half: out = hi*cos_h + lo*sin_h
                t3 = tmp.tile([P, FB], f32, tag="t", name=f"t3{dlow}{nb}")
                nc.vector.tensor_mul(t3, ps_hi, cos_h)
                t4 = tmp.tile([P, FB], f32, tag="t", name=f"t4{dlow}{nb}")
                nc.vector.tensor_mul(t4, ps_lo, sin_h)
                o_hi = out_ap(dhigh, nb)
                nc.vector.tensor_add(o_hi, t3, t4)
                if post is not None:
                    post(dhigh, nb, o_hi)

            # ---- K projection + rope -> kt_local; single gather after ----
            def k_out(dc, nb):
                return strm.tile([P, FB], bf16, tag="ro", name=f"kt_{dc}_{nb}")

            def k_post(dc, nb, t):
                # contiguous 128KB store; the slab loads do the chunk-row
                # transpose on their (dram-side, unconstrained) access pattern
                nc.gpsimd.dma_start(out=kt_local[dc, nb], in_=t)

            wv_pool = tc.alloc_tile_pool(name="wv_pool", bufs=2)

            def wv_load(wb):
                t = wv_pool.tile([P, KD, FB], bf16, tag="wv", name=f"wv{wb}")
                nc.sync.dma_start(out=t[:, : KD // 2, :], in_=wv_ext[wb, :, : KD // 2, :])
                nc.scalar.dma_start(
                    out=t[:, KD // 2 :, :], in_=wv_ext[wb, :, KD // 2 :, :]
                )
                return t

            with tc.tile_pool(name="wk_pool", bufs=4) as wkp:
                # first-needed halves first: x columns 0:512 feed the nb=0
                # chains, so the first K unit starts after ~2.5MB not 4.5MB
                for kg in range(8):
                    x_dma_engines[kg].dma_start(
                        out=x_ts[kg][:, :, :FB],
                        in_=x_ext[:, kg * 2 : (kg + 1) * 2, :FB],
                    )
                pre = [load_panels(wkp, wk_ext, d) for d in range(2)]
                for dc in (0, KD // 2, 1, KD // 2 + 1):
                    nc.sync.dma_start(out=cos_sb[:, dc, :], in_=cos_ext[dc])
                    nc.scalar.dma_start(out=sin_sb[:, dc, :], in_=sin_ext[dc])
                for kg in range(8):
                    x_dma_engines[kg].dma_start(
                        out=x_ts[kg][:, :, FB:],
                        in_=x_ext[:, kg * 2 : (kg + 1) * 2, FB:],
                    )
                for dlow in range(2, KD // 2):
                    for dc in (dlow, dlow + KD // 2):
                        nc.sync.dma_start(out=cos_sb[:, dc, :], in_=cos_ext[dc])
                        nc.scalar.dma_start(out=sin_sb[:, dc, :], in_=sin_ext[dc])
                wv0 = None
                for dlow in range(KD // 2):
                    panels = (
                        pre[dlow] if dlow < 2 else load_panels(wkp, wk_ext, dlow)
                    )
                    if dlow == 6:
                        wv0 = wv_load(0)  # prefetch first V block
                    for nb in range(NB):
                        rope_pair(panels, dlow, nb, k_out, k_post)
                for half in range(2):
                    sl = slice(half * KD // 2, (half + 1) * KD // 2)
                    nc.gpsimd.collective_compute(
                        "AllGather",
                        mybir.AluOpType.bypass,
                        replica_groups=PAIRS,
                        ins=[kt_local[sl].opt()],
                        outs=[(kt_gath_a if half == 0 else kt_gath_b).opt()],
                    )

            # ---- V projection (wb-outer: wv streamed per dout block) ----
            for wb in range(4):
                wv_t = wv0 if wb == 0 else wv_load(wb)
                half, col = wb // 2, (wb % 2) * FB
                for ncc in range(NCJ // 2):
                    ps_v = psum.tile([P, FB], f32, tag="ps")
                    for k in range(KD):
                        nc.tensor.matmul(
                            ps_v,
                            lhsT=x_ref(k)[:, ncc * P : (ncc + 1) * P],
                            rhs=wv_t[:, k, :],
                            start=(k == 0),
                            stop=(k == KD - 1),
                        )
                    v_t = strm.tile([P, FB], bf16, tag="vo")
                    nc.vector.tensor_copy(v_t, ps_v)
                    _panel_engines[ncc % 2].dma_start(
                        out=v_local[half, ncc][:, col : col + FB], in_=v_t
                    )
            # both V gathers at V end: the collective wait blocks the gpsimd
            # queue, so nothing latency-critical may sit behind it
            for half in range(2):
                nc.gpsimd.collective_compute(
                    "AllGather",
                    mybir.AluOpType.bypass,
                    replica_groups=PAIRS,
                    ins=[v_local[half].opt()],
                    outs=[v_gath[half].opt()],
                )
            wv_pool.release()

            # ---- Q projection + rope (cos/sin reused from SBUF) ----
            def q_out(dc, nb):
                return qt_sb[:, dc, nb * FB : (nb + 1) * FB]

            with tc.tile_pool(name="wq_pool", bufs=4) as wqp:
                for dlow in range(KD // 2):
                    panels = load_panels(wqp, wq_ext, dlow)
                    for nb in range(NB):
                        rope_pair(panels, dlow, nb, q_out, None)
            # junk matmuls bridge the Q-tail DVE drain so the PE activity
            # monitor does not re-throttle right before the S phase
            for i in range(12):
                ps_f = dnsum.tile([P, QW], f32, tag="rb", name=f"fill{i}")
                nc.tensor.matmul(
                    ps_f, lhsT=wu_sb[:, :P], rhs=wu_sb[:, :QW], start=True, stop=True
                )
            x_pool.release()
            cs_pool.release()

            # ---- Attention ----
            with (
                tc.tile_pool(name="v2_pool", bufs=1) as v2p,
                tc.tile_pool(name="pt_pool", bufs=1) as ptp,
                tc.tile_pool(name="slab", bufs=6) as slab,
                tc.tile_pool(name="mskp", bufs=3) as mskp,
                tc.tile_pool(name="outp", bufs=3) as outp,
                tc.tile_pool(name="smallp", bufs=2) as smallp,
            ):
                # v2 in 4 independent tiles so the chunk DMAs run in parallel
                v2_ts = [
                    v2p.tile([P, 4, D], bf16, name=f"v2_sb{i}") for i in range(4)
                ]

                def v2_ref(jc):
                    return v2_ts[jc // 4][:, jc % 4, :]

                _slab_engines = [nc.sync, nc.scalar]
                slabs = {}

                def load_slab(jc):
                    t = slab.tile([P, KD, P], bf16, tag="slab", name=f"slab{jc}")
                    e = _slab_engines[jc % 2]
                    e.dma_start(out=t[:, : KD // 2, :], in_=kt_g(jc, 0))
                    e.dma_start(out=t[:, KD // 2 :, :], in_=kt_g(jc, 1))
                    return t

                # ALL slab DMAs ahead of the v2 bulk: the v2 loads wait on the
                # V gathers, and queued-behind slab DMAs would stall S with them
                for jc in range(6):
                    slabs[jc] = load_slab(jc)

                # v2 chunk loads ordered by first use (onesum stages first),
                # all on gpsimd so they never block slab/mask/out DMAs on the
                # HWDGE queues while waiting for the V gathers
                v2_order = [jc for st in OS_STAGES for jc in st]
                v2_order += [jc for jc in range(NCJ) if jc not in v2_order]
                for jc in v2_order:
                    for h in range(2):
                        nc.gpsimd.dma_start(
                            out=v2_ref(jc)[:, h * NOWN : (h + 1) * NOWN],
                            in_=v_gath[h, jc // 8, jc % 8],
                        )

                pt_sb = ptp.tile([P, NCJ, NOWN], bf16)

                def s_tile(jc, q, kt_slab, msk):
                    sl = slice(q * QW, (q + 1) * QW)
                    ps_s = psum.tile([P, QW], f32, tag="ps", name=f"ps_s{jc}{q}")
                    for k in range(KD):
                        nc.tensor.matmul(
                            ps_s,
                            lhsT=kt_slab[:, k, :],
                            rhs=qt_sb[:, k, sl],
                            start=(k == 0),
                            stop=(k == KD - 1),
                        )
                    if msk is not None:
                        tm = tmp.tile([P, QW], f32, tag="t", name=f"tm{jc}{q}")
                        nc.vector.tensor_mul(tm, ps_s, msk)
                        esrc = tm
                    else:
                        esrc = ps_s
                    nc.scalar.activation(
                        out=pt_sb[:, jc, sl],
                        in_=esrc,
                        func=mybir.ActivationFunctionType.Exp,
                        scale=SCALE,
                    )

                # onesum partials per stage (interleaved into the S phase so
                # the PE activity monitor never sees an idle window), then
                # cumulative sums on DVE: q2 uses p0, q1 p0+p1, q0 p0+p1+p2
                os_parts = []

                def emit_os_stage(si):
                    stage = OS_STAGES[si]
                    ps_os = dnsum.tile([P, KD], f32, tag="os", name=f"pso_{si}")
                    for dc in range(KD):
                        for idx, jc in enumerate(stage):
                            nc.tensor.matmul(
                                ps_os[:, dc : dc + 1],
                                lhsT=v2_ref(jc)[:, dc * P : (dc + 1) * P],
                                rhs=ones_col,
                                start=(idx == 0),
                                stop=(idx == len(stage) - 1),
                            )
                    p_sb = smallp.tile([P, KD], f32, tag=f"osp{si}", name=f"osp{si}")
                    nc.vector.tensor_copy(p_sb, ps_os)
                    os_parts.append(p_sb)

                for jc in range(NCJ):
                    if jc + 6 < NCJ:
                        slabs[jc + 6] = load_slab(jc + 6)
                    kt_slab = slabs.pop(jc)
                    msk = mskp.tile([P, QW], bf16, tag="m")
                    nc.scalar.dma_start(out=msk, in_=mask_ext[jc])
                    m = _m_of(jc)
                    s_tile(jc, m, kt_slab, msk)  # the mixed quarter
                    for q in range(m + 1, NQ):
                        s_tile(jc, q, kt_slab, None)  # fully unmasked
                    if jc in (10, 13, 15):
                        emit_os_stage({10: 0, 13: 1, 15: 2}[jc])

                os2 = os_parts[0]
                os1 = smallp.tile([P, KD], f32, tag="os1c")
                nc.vector.tensor_add(os1, os_parts[0], os_parts[1])
                os0 = smallp.tile([P, KD], f32, tag="os0c")
                nc.vector.tensor_add(os0, os1, os_parts[2])
                os_of_q = {2: os2, 1: os1, 0: os0}

                # denominators + reciprocals + their broadcast; skipped chunks
                # contribute (12 - 4q)*128 exact ones
                rbs = {}
                for q in range(NQ):
                    jcs = _comp(q)
                    ps_d = dnsum.tile([1, QW], f32, tag="dn", name=f"psd{q}")
                    for idx, jc in enumerate(jcs):
                        nc.tensor.matmul(
                            ps_d,
                            lhsT=ones_col,
                            rhs=pt_sb[:, jc, q * QW : (q + 1) * QW],
                            start=(idx == 0),
                            stop=(idx == len(jcs) - 1),
                        )
                    recip = smallp.tile([1, QW], f32, tag="rc", name=f"rc{q}")
                    nones = (12 - 4 * q) * P
                    if nones:
                        dfix = smallp.tile([1, QW], f32, tag="dfix", name=f"df{q}")
                        nc.vector.tensor_scalar_add(dfix, ps_d, float(nones))
                        nc.vector.reciprocal(recip, dfix)
                    else:
                        nc.vector.reciprocal(recip, ps_d)
                    # reciprocal broadcast via fp32 outer product
                    ps_rb = dnsum.tile([P, QW], f32, tag="rb", name=f"prb{q}")
                    nc.tensor.matmul(
                        ps_rb, lhsT=ones_row, rhs=recip, start=True, stop=True
                    )
                    rb = smallp.tile([P, QW], f32, tag=f"rbs{q}", name=f"rb{q}")
                    nc.vector.tensor_copy(rb, ps_rb)
                    rbs[q] = rb

                # ---- PV, dout-major so each dc finishes as one 512KB DMA ----
                _out_engines = [nc.sync, nc.scalar]
                for dc in range(KD):
                    o_st = outp.tile([P, NOWN], f32, tag="o", name=f"ost{dc}")
                    for q in range(NQ):
                        jcs = _comp(q)
                        ps_o = psum.tile([P, QW], f32, tag="ps", name=f"pso{q}{dc}")
                        for idx, jc in enumerate(jcs):
                            nc.tensor.matmul(
                                ps_o,
                                lhsT=v2_ref(jc)[:, dc * P : (dc + 1) * P],
                                rhs=pt_sb[:, jc, q * QW : (q + 1) * QW],
                                start=(idx == 0),
                                stop=(idx == len(jcs) - 1),
                            )
                        osl = o_st[:, q * QW : (q + 1) * QW]
                        if q in os_of_q:
                            nc.vector.scalar_tensor_tensor(
                                out=osl,
                                in0=ps_o,
                                scalar=os_of_q[q][:, dc : dc + 1],
                                in1=rbs[q],
                                op0=mybir.AluOpType.add,
                                op1=mybir.AluOpType.mult,
                            )
                        else:
                            nc.vector.tensor_mul(osl, ps_o, rbs[q])
                    _out_engines[dc % 2].dma_start(
                        out=out_ext[dc * P : (dc + 1) * P, :], in_=o_st
                    )

    nc.compile()
    return nc


def _prep_inputs(x, cos, sin, Wq, Wk, Wv):
    """Host-side sharding/layout prep. Returns in_maps for 8 cores."""
    x = np.asarray(x, dtype=np.float32)
    cos = np.asarray(cos, dtype=np.float32)
    sin = np.asarray(sin, dtype=np.float32)

    def w_panels(w):
        # W.T [din, dout] -> [dc, p_din, k_din, c_dout] with d = k*128+p
        wt = np.ascontiguousarray(np.asarray(w, dtype=np.float32).T).astype(BF16)
        return np.ascontiguousarray(
            wt.reshape(KD, P, KD, P).transpose(2, 1, 0, 3)
        )

    wq_p = w_panels(Wq)
    wk_p = w_panels(Wk)
    # Wv.T [din, dout] -> [wb, p, k, 512] (dout blocks contiguous)
    wv_p = np.ascontiguousarray(
        np.asarray(Wv, dtype=np.float32)
        .T.astype(BF16)
        .reshape(KD, P, 4, FB)
        .transpose(2, 1, 0, 3)
    )

    # global row index of gathered slot s: pair rank h2 = s // NOWN owns the
    # rows with parity h2, so j_global(s) = 2*(s % NOWN) + h2
    slot = np.arange(S, dtype=np.int64)
    j_global = 2 * (slot % NOWN) + slot // NOWN

    in_maps = []
    for c in range(N_CORES):
        b, h = divmod(c, 2)
        rows = slice(h, None, 2)  # interleaved rows: h, h+2, h+4, ...
        xt = np.ascontiguousarray(
            x[b, rows, :].T.astype(BF16).reshape(KD, P, NOWN).transpose(1, 0, 2)
        )
        cos_t = np.ascontiguousarray(cos[rows].T.astype(BF16).reshape(KD, P, NOWN))
        sin_t = np.ascontiguousarray(sin[rows].T.astype(BF16).reshape(KD, P, NOWN))
        i_global = 2 * np.arange(NOWN, dtype=np.int64) + h
        # per jc, only the "mixed" i-quarter needs mask data
        mask_t = np.empty((NCJ, P, QW), dtype=BF16)
        for jc in range(NCJ):
            q = (jc % 8) // 2
            jg = j_global[jc * P : (jc + 1) * P][:, None]
            ig = i_global[q * QW : (q + 1) * QW][None, :]
            mask_t[jc] = (jg <= ig).astype(BF16)
        in_maps.append(
            {
                "x_t": xt,
                "wq": wq_p,
                "wk": wk_p,
                "wv": wv_p,
                "cos_t": cos_t,
                "sin_t": sin_t,
                "mask_t": mask_t,
            }
        )
    return in_maps


def _run(in_maps, trace=False, tmpdir=None):
    from concourse.bass_utils import run_bass_kernel_spmd

    if "nc" not in _CACHE:
        _CACHE["nc"] = _build()
    nc = _CACHE["nc"]
    return run_bass_kernel_spmd(
        nc, in_maps, list(range(N_CORES)), trace=trace, tmpdir=tmpdir
    )


def kernel(x, cos, sin, Wq, Wk, Wv):
    in_maps = _prep_inputs(x, cos, sin, Wq, Wk, Wv)
    res = _run(in_maps, trace=False)
    out = np.empty((B, S, D), dtype=np.float32)
    for c in range(N_CORES):
        b, h = divmod(c, 2)
        out[b, h::2, :] = res.results[c]["out"].T
    return out


# revision 21
# speedup vs baseline: 1.1874x; 1.1874x over previous
"""Trainium2 Bass kernel: single-head attention with RoPE and the reference's
multiplicative causal mask (masked logits stay 0 -> exp(0)=1, so masked
positions contribute exp(0)=1 to softmax -- attention is dense over the
upper triangle too, but those probabilities are a constant 1/Z).

Sharding: 8 cores = 4 batches x 2 row-parity halves. Core (b, h) owns the
interleaved rows x[b, h::2] -- with this split the causal-mask tile classes
are identical on every core, so fully-masked S^T tiles are skipped
statically (same SPMD graph everywhere) and their P==1 contribution enters
as a per-dout constant (onesum) plus a denominator offset.

Per core: project K (dlow-outer so each weight panel loads once), RoPE
on-chip, single AllGather of roped K within the 2-core pair; V projection
streamed by dout block (wv loaded per block, not upfront) with the gather
split in two dout halves; Q projection reusing the cos/sin tables kept in
SBUF since the K phase; then S^T = K@Q^T, P = exp(mask*S^T/sqrt(S)) with
the onesum chains interleaved to keep the PE clock-gate warm, and
O^T = V^T@P^T / denom emitted dout-major so outputs stream back in 512KB
DMAs. Output is O^T per core; the host transposes and reassembles.
"""

import sys

for _p in ("/opt/trn_rl_repo", "/root/.axon_site/_ro/trn_rl_repo"):
    if _p not in sys.path:
        sys.path.append(_p)

import math

import ml_dtypes
import numpy as np

BF16 = ml_dtypes.bfloat16

B, S, D = 4, 2048, 2048
NOWN = 1024  # query rows per core
P = 128  # partitions
KD = D // P  # 16 feature chunks
NCJ = S // P  # 16 key chunks
N_CORES = 8
PAIRS = [[0, 1], [2, 3], [4, 5], [6, 7]]
FB = 512  # matmul moving free-dim block
NB = NOWN // FB  # 2 blocks of own rows
SCALE = 1.0 / math.sqrt(S)  # reference scales by sqrt(seq_len), not sqrt(D)

# Quarter-granularity mask staircase (identical on every core with
# interleaved rows): for i-quarter q (256 columns) and j-chunk jc with
# m = (jc % 8) // 2:  q < m -> fully masked (skipped, P==1);
# q == m -> mixed (mask applied); q > m -> fully unmasked.
QW = 256
NQ = NOWN // QW  # 4 quarters


def _m_of(jc):
    return (jc % 8) // 2


# chunks contributing computed S tiles for quarter q
def _comp(q):
    return [jc for jc in range(NCJ) if _m_of(jc) <= q]


# onesum stages: chunks that become skipped when stepping down a quarter
OS_STAGES = [
    [jc for jc in range(NCJ) if _m_of(jc) == 3],  # skipped for q<=2
    [jc for jc in range(NCJ) if _m_of(jc) == 2],  # additionally for q<=1
    [jc for jc in range(NCJ) if _m_of(jc) == 1],  # additionally for q==0
]

_CACHE = {}


def _build():
    import concourse.bass as bass  # noqa: F401
    import concourse.tile as tile
    from concourse import bacc, mybir

    f32 = mybir.dt.float32
    bf16 = mybir.dt.bfloat16

    nc = bacc.Bacc(
        "TRN2", target_bir_lowering=False, debug=False, num_devices=N_CORES
    )

    x_ext = nc.dram_tensor("x_t", [P, KD, NOWN], bf16, kind="ExternalInput").ap()
    wq_ext = nc.dram_tensor("wq", [KD, P, KD, P], bf16, kind="ExternalInput").ap()
    wk_ext = nc.dram_tensor("wk", [KD, P, KD, P], bf16, kind="ExternalInput").ap()
    # wv re-laid host-side as [wb, p, k, 512] so each dout block is contiguous
    wv_ext = nc.dram_tensor("wv", [4, P, KD, FB], bf16, kind="ExternalInput").ap()
    cos_ext = nc.dram_tensor("cos_t", [KD, P, NOWN], bf16, kind="ExternalInput").ap()
    sin_ext = nc.dram_tensor("sin_t", [KD, P, NOWN], bf16, kind="ExternalInput").ap()
    mask_ext = nc.dram_tensor("mask_t", [NCJ, P, QW], bf16, kind="ExternalInput").ap()
    out_ext = nc.dram_tensor("out", [D, NOWN], f32, kind="ExternalOutput").ap()

    with tile.TileContext(nc) as tc:
        with (
            tc.tile_pool(name="dram", bufs=1, space="DRAM") as dram,
            tc.tile_pool(name="psum", bufs=5, space="PSUM") as psum,
            tc.tile_pool(name="dnsum", bufs=1, space="PSUM") as dnsum,
            tc.tile_pool(name="persist", bufs=1) as persist,
            tc.tile_pool(name="tmp", bufs=6) as tmp,
            tc.tile_pool(name="strm", bufs=8) as strm,
        ):
            kt_local = dram.tile([NCJ // 2, P, KD, P], bf16)
            v_local = dram.tile([2, NCJ // 2, P, NOWN], bf16)  # dout halves
            kt_gath_a = dram.tile([2, 4, P, KD, P], bf16)
            kt_gath_b = dram.tile([2, 4, P, KD, P], bf16)
            v_gath = dram.tile([2, 2, NCJ // 2, P, NOWN], bf16)

            def kt_g(jc):
                r, loc = jc // 8, jc % 8
                g = kt_gath_a if loc < 4 else kt_gath_b
                return g[r, loc % 4]

            ones_col = persist.tile([P, 1], bf16)
            nc.vector.memset(ones_col, 1.0)
            ones_row = persist.tile([P, P], f32)
            nc.vector.memset(ones_row, 1.0)
            # +nones bias rows for the packed denominators (row 32q holds
            # quarter q's count of statically-skipped ones)
            dbias = persist.tile([P, QW], f32, name="dbias")
            nc.vector.memset(dbias, 0.0)
            for q in range(NQ - 1):
                nc.vector.memset(dbias[32 * q : 32 * q + 1, :], float((12 - 4 * q) * P))

            qt_sb = persist.tile([P, KD, NOWN], bf16)

            # warm-up matmuls on a memset tile: ~5us of PE activity during the
            # initial DMA window flips the HAM clock-gate to 2.4GHz before the
            # first real chain issues (otherwise the first ~3.4us run at half
            # clock), at zero cost since the PE would be idle anyway
            wu_sb = persist.tile([P, FB], bf16, name="wu_sb")
            nc.vector.memset(wu_sb, 0.0)
            for i in range(12):
                ps_w = psum.tile([P, FB], f32, tag="ps", name=f"wup{i}")
                nc.tensor.matmul(
                    ps_w, lhsT=wu_sb[:, :P], rhs=wu_sb, start=True, stop=True
                )

            # ---- staged loads: weights dlow=0 first, then x, then cos/sin ----
            cs_pool = tc.alloc_tile_pool(name="cs_pool", bufs=1)
            cos_sb = cs_pool.tile([P, KD, NOWN], bf16, name="cos_sb")
            sin_sb = cs_pool.tile([P, KD, NOWN], bf16, name="sin_sb")

            # x in 8 independent tiles so the chunk DMAs run in parallel
            x_pool = tc.alloc_tile_pool(name="x_pool", bufs=1)
            x_ts = [
                x_pool.tile([P, 2, NOWN], bf16, name=f"x_sb{i}") for i in range(8)
            ]
            x_dma_engines = [
                nc.sync,
                nc.scalar,
                nc.sync,
                nc.scalar,
                nc.sync,
                nc.scalar,
                nc.sync,
                nc.scalar,
            ]

            def x_ref(k):
                return x_ts[k // 2][:, k % 2, :]

            _panel_engines = [nc.sync, nc.scalar]

            def load_panels(wpool, w_ext, dlow):
                dhigh = dlow + KD // 2
                e0 = _panel_engines[dlow % 2]
                e1 = _panel_engines[(dlow + 1) % 2]
                w_lo = wpool.tile([P, KD, P], bf16, tag="wp", name=f"wlo{dlow}")
                e0.dma_start(out=w_lo, in_=w_ext[dlow])
                w_hi = wpool.tile([P, KD, P], bf16, tag="wp", name=f"whi{dlow}")
                e1.dma_start(out=w_hi, in_=w_ext[dhigh])
                return w_lo, w_hi

            def rope_pair(panels, dlow, nb, out_ap, post):
                """One (dlow, nb) unit: two projection chains + rope."""
                dhigh = dlow + KD // 2
                sl = slice(nb * FB, (nb + 1) * FB)
                cos_t = cos_sb[:, dlow, sl]
                sin_t = sin_sb[:, dlow, sl]
                cos_h = cos_sb[:, dhigh, sl]
                sin_h = sin_sb[:, dhigh, sl]
                w_lo, w_hi = panels
                ps_lo = psum.tile([P, FB], f32, tag="ps", name=f"plo{dlow}{nb}")
                for k in range(KD):
                    nc.tensor.matmul(
                        ps_lo,
                        lhsT=w_lo[:, k, :],
                        rhs=x_ref(k)[:, sl],
                        start=(k == 0),
                        stop=(k == KD - 1),
                    )
                ps_hi = psum.tile([P, FB], f32, tag="ps", name=f"phi{dlow}{nb}")
                for k in range(KD):
                    nc.tensor.matmul(
                        ps_hi,
                        lhsT=w_hi[:, k, :],
                        rhs=x_ref(k)[:, sl],
                        start=(k == 0),
                        stop=(k == KD - 1),
                    )
                # rope low half: out = lo*cos_l - hi*sin_l
                t1 = tmp.tile([P, FB], f32, tag="t", name=f"t1{dlow}{nb}")
                nc.vector.tensor_mul(t1, ps_lo, cos_t)
                t2 = tmp.tile([P, FB], f32, tag="t", name=f"t2{dlow}{nb}")
                nc.vector.tensor_mul(t2, ps_hi, sin_t)
                o_lo = out_ap(dlow, nb)
                nc.vector.tensor_sub(o_lo, t1, t2)
                if post is not None:
                    post(dlow, nb, o_lo)
                # rope high half: out = hi*cos_h + lo*sin_h
                t3 = tmp.tile([P, FB], f32, tag="t", name=f"t3{dlow}{nb}")
                nc.vector.tensor_mul(t3, ps_hi, cos_h)
                t4 = tmp.tile([P, FB], f32, tag="t", name=f"t4{dlow}{nb}")
                nc.vector.tensor_mul(t4, ps_lo, sin_h)
                o_hi = out_ap(dhigh, nb)
                nc.vector.tensor_add(o_hi, t3, t4)
                if post is not None:
                    post(dhigh, nb, o_hi)

            # ---- K projection + rope -> kt_local; single gather after ----
            def k_out(dc, nb):
                return strm.tile([P, FB], bf16, tag="ro", name=f"kt_{dc}_{nb}")

            def k_post(dc, nb, t):
                for jj in range(FB // P):
                    nc.gpsimd.dma_start(
                        out=kt_local[nb * 4 + jj][:, dc, :],
                        in_=t[:, jj * P : (jj + 1) * P],
                    )

            wv_pool = tc.alloc_tile_pool(name="wv_pool", bufs=2)

            def wv_load(wb):
                t = wv_pool.tile([P, KD, FB], bf16, tag="wv", name=f"wv{wb}")
                nc.sync.dma_start(out=t[:, : KD // 2, :], in_=wv_ext[wb, :, : KD // 2, :])
                nc.scalar.dma_start(
                    out=t[:, KD // 2 :, :], in_=wv_ext[wb, :, KD // 2 :, :]
                )
                return t

            with tc.tile_pool(name="wk_pool", bufs=4) as wkp:
                for kg in (0, 1):
                    x_dma_engines[kg].dma_start(
                        out=x_ts[kg], in_=x_ext[:, kg * 2 : (kg + 1) * 2, :]
                    )
                pre = [load_panels(wkp, wk_ext, d) for d in range(2)]
                for kg in range(2, 8):
                    x_dma_engines[kg].dma_start(
                        out=x_ts[kg], in_=x_ext[:, kg * 2 : (kg + 1) * 2, :]
                    )
                # cos/sin pieces ordered by first use: dc = 0,8,1,9,2,10,...
                for dlow in range(KD // 2):
                    for dc in (dlow, dlow + KD // 2):
                        nc.sync.dma_start(out=cos_sb[:, dc, :], in_=cos_ext[dc])
                        nc.scalar.dma_start(out=sin_sb[:, dc, :], in_=sin_ext[dc])
                wv0 = None
                for dlow in range(KD // 2):
                    panels = (
                        pre[dlow] if dlow < 2 else load_panels(wkp, wk_ext, dlow)
                    )
                    if dlow == 6:
                        wv0 = wv_load(0)  # prefetch first V block
                    for nb in range(NB):
                        rope_pair(panels, dlow, nb, k_out, k_post)
                for half in range(2):
                    sl = slice(half * 4, (half + 1) * 4)
                    nc.gpsimd.collective_compute(
                        "AllGather",
                        mybir.AluOpType.bypass,
                        replica_groups=PAIRS,
                        ins=[kt_local[sl].opt()],
                        outs=[(kt_gath_a if half == 0 else kt_gath_b).opt()],
                    )

            # ---- V projection (wb-outer: wv streamed per dout block) ----
            for wb in range(4):
                wv_t = wv0 if wb == 0 else wv_load(wb)
                half, col = wb // 2, (wb % 2) * FB
                for ncc in range(NCJ // 2):
                    ps_v = psum.tile([P, FB], f32, tag="ps")
                    for k in range(KD):
                        nc.tensor.matmul(
                            ps_v,
                            lhsT=x_ref(k)[:, ncc * P : (ncc + 1) * P],
                            rhs=wv_t[:, k, :],
                            start=(k == 0),
                            stop=(k == KD - 1),
                        )
                    v_t = strm.tile([P, FB], bf16, tag="vo")
                    nc.vector.tensor_copy(v_t, ps_v)
                    _panel_engines[ncc % 2].dma_start(
                        out=v_local[half, ncc][:, col : col + FB], in_=v_t
                    )
            # both V gathers at V end: the collective wait blocks the gpsimd
            # queue, so nothing latency-critical may sit behind it
            for half in range(2):
                nc.gpsimd.collective_compute(
                    "AllGather",
                    mybir.AluOpType.bypass,
                    replica_groups=PAIRS,
                    ins=[v_local[half].opt()],
                    outs=[v_gath[half].opt()],
                )
            wv_pool.release()

            # ---- Q projection + rope (cos/sin reused from SBUF) ----
            def q_out(dc, nb):
                return qt_sb[:, dc, nb * FB : (nb + 1) * FB]

            with tc.tile_pool(name="wq_pool", bufs=4) as wqp:
                for dlow in range(KD // 2):
                    panels = load_panels(wqp, wq_ext, dlow)
                    for nb in range(NB):
                        rope_pair(panels, dlow, nb, q_out, None)
            # junk matmuls bridge the Q-tail DVE drain so the PE activity
            # monitor does not re-throttle right before the S phase
            for i in range(12):
                ps_f = dnsum.tile([P, QW], f32, tag="rb", name=f"fill{i}")
                nc.tensor.matmul(
                    ps_f, lhsT=wu_sb[:, :P], rhs=wu_sb[:, :QW], start=True, stop=True
                )
            x_pool.release()
            cs_pool.release()

            # ---- Attention ----
            with (
                tc.tile_pool(name="v2_pool", bufs=1) as v2p,
                tc.tile_pool(name="pt_pool", bufs=1) as ptp,
                tc.tile_pool(name="slab", bufs=6) as slab,
                tc.tile_pool(name="mskp", bufs=3) as mskp,
                tc.tile_pool(name="outp", bufs=3) as outp,
                tc.tile_pool(name="smallp", bufs=1) as smallp,
            ):
                # v2 in 4 independent tiles so the chunk DMAs run in parallel
                v2_ts = [
                    v2p.tile([P, 4, D], bf16, name=f"v2_sb{i}") for i in range(4)
                ]

                def v2_ref(jc):
                    return v2_ts[jc // 4][:, jc % 4, :]

                _slab_engines = [nc.sync, nc.scalar]
                slabs = {}

                def load_slab(jc):
                    t = slab.tile([P, KD, P], bf16, tag="slab", name=f"slab{jc}")
                    _slab_engines[jc % 2].dma_start(out=t, in_=kt_g(jc))
                    return t

                # ALL slab DMAs ahead of the v2 bulk: the v2 loads wait on the
                # V gathers, and queued-behind slab DMAs would stall S with them
                for jc in range(6):
                    slabs[jc] = load_slab(jc)

                # v2 chunk loads ordered by first use (onesum stages first),
                # all on gpsimd so they never block slab/mask/out DMAs on the
                # HWDGE queues while waiting for the V gathers
                v2_order = [jc for st in OS_STAGES for jc in st]
                v2_order += [jc for jc in range(NCJ) if jc not in v2_order]
                for jc in v2_order:
                    for h in range(2):
                        nc.gpsimd.dma_start(
                            out=v2_ref(jc)[:, h * NOWN : (h + 1) * NOWN],
                            in_=v_gath[h, jc // 8, jc % 8],
                        )

                pt_sb = ptp.tile([P, NCJ, NOWN], bf16)

                def s_tile(jc, q, kt_slab, msk):
                    sl = slice(q * QW, (q + 1) * QW)
                    ps_s = psum.tile([P, QW], f32, tag="ps", name=f"ps_s{jc}{q}")
                    for k in range(KD):
                        nc.tensor.matmul(
                            ps_s,
                            lhsT=kt_slab[:, k, :],
                            rhs=qt_sb[:, k, sl],
                            start=(k == 0),
                            stop=(k == KD - 1),
                        )
                    if msk is not None:
                        tm = tmp.tile([P, QW], f32, tag="t", name=f"tm{jc}{q}")
                        nc.vector.tensor_mul(tm, ps_s, msk)
                        esrc = tm
                    else:
                        esrc = ps_s
                    nc.scalar.activation(
                        out=pt_sb[:, jc, sl],
                        in_=esrc,
                        func=mybir.ActivationFunctionType.Exp,
                        scale=SCALE,
                    )

                # onesum partials per stage (interleaved into the S phase so
                # the PE activity monitor never sees an idle window), then
                # cumulative sums on DVE: q2 uses p0, q1 p0+p1, q0 p0+p1+p2
                os_parts = []

                def emit_os_stage(si):
                    stage = OS_STAGES[si]
                    ps_os = dnsum.tile([P, KD], f32, tag="os", name=f"pso_{si}")
                    for dc in range(KD):
                        for idx, jc in enumerate(stage):
                            nc.tensor.matmul(
                                ps_os[:, dc : dc + 1],
                                lhsT=v2_ref(jc)[:, dc * P : (dc + 1) * P],
                                rhs=ones_col,
                                start=(idx == 0),
                                stop=(idx == len(stage) - 1),
                            )
                    p_sb = smallp.tile([P, KD], f32, tag=f"osp{si}", name=f"osp{si}")
                    nc.vector.tensor_copy(p_sb, ps_os)
                    os_parts.append(p_sb)

                for jc in range(NCJ):
                    if jc + 6 < NCJ:
                        slabs[jc + 6] = load_slab(jc + 6)
                    kt_slab = slabs.pop(jc)
                    msk = mskp.tile([P, QW], bf16, tag="m")
                    nc.scalar.dma_start(out=msk, in_=mask_ext[jc])
                    m = _m_of(jc)
                    s_tile(jc, m, kt_slab, msk)  # the mixed quarter
                    for q in range(m + 1, NQ):
                        s_tile(jc, q, kt_slab, None)  # fully unmasked
                    if jc in (10, 13, 15):
                        emit_os_stage({10: 0, 13: 1, 15: 2}[jc])

                os2 = os_parts[0]
                os1 = smallp.tile([P, KD], f32, tag="os1c")
                nc.vector.tensor_add(os1, os_parts[0], os_parts[1])
                os0 = smallp.tile([P, KD], f32, tag="os0c")
                nc.vector.tensor_add(os0, os1, os_parts[2])
                os_of_q = {2: os2, 1: os1, 0: os0}

                # denominators + reciprocals + their broadcast; skipped chunks
                # contribute (12 - 4q)*128 exact ones
                rbs = {}
                for q in range(NQ):
                    jcs = _comp(q)
                    ps_d = dnsum.tile([1, QW], f32, tag="dn", name=f"psd{q}")
                    for idx, jc in enumerate(jcs):
                        nc.tensor.matmul(
                            ps_d,
                            lhsT=ones_col,
                            rhs=pt_sb[:, jc, q * QW : (q + 1) * QW],
                            start=(idx == 0),
                            stop=(idx == len(jcs) - 1),
                        )
                    recip = smallp.tile([1, QW], f32, tag="rc", name=f"rc{q}")
                    nones = (12 - 4 * q) * P
                    if nones:
                        dfix = smallp.tile([1, QW], f32, tag="dfix", name=f"df{q}")
                        nc.vector.tensor_scalar_add(dfix, ps_d, float(nones))
                        nc.vector.reciprocal(recip, dfix)
                    else:
                        nc.vector.reciprocal(recip, ps_d)
                    # reciprocal broadcast via fp32 outer product
                    ps_rb = dnsum.tile([P, QW], f32, tag="rb", name=f"prb{q}")
                    nc.tensor.matmul(
                        ps_rb,
                        lhsT=ones_row[:1, :],
                        rhs=recip,
                        start=True,
                        stop=True,
                    )
                    rb = smallp.tile([P, QW], f32, tag=f"rbs{q}", name=f"rb{q}")
                    nc.vector.tensor_copy(rb, ps_rb)
                    rbs[q] = rb

                # ---- PV, dout-major so each dc finishes as one 512KB DMA ----
                _out_engines = [nc.sync, nc.scalar]
                for dc in range(KD):
                    o_st = outp.tile([P, NOWN], f32, tag="o", name=f"ost{dc}")
                    for q in range(NQ):
                        jcs = _comp(q)
                        ps_o = psum.tile([P, QW], f32, tag="ps", name=f"pso{q}{dc}")
                        for idx, jc in enumerate(jcs):
                            nc.tensor.matmul(
                                ps_o,
                                lhsT=v2_ref(jc)[:, dc * P : (dc + 1) * P],
                                rhs=pt_sb[:, jc, q * QW : (q + 1) * QW],
                                start=(idx == 0),
                                stop=(idx == len(jcs) - 1),
                            )
                        osl = o_st[:, q * QW : (q + 1) * QW]
                        if q in os_of_q:
                            nc.vector.scalar_tensor_tensor(
                                out=osl,
                                in0=ps_o,
                                scalar=os_of_q[q][:, dc : dc + 1],
                                in1=rbs[q],
                                op0=mybir.AluOpType.add,
                                op1=mybir.AluOpType.mult,
                            )
                        else:
                            nc.vector.tensor_mul(osl, ps_o, rbs[q])
                    _out_engines[dc % 2].dma_start(
                        out=out_ext[dc * P : (dc + 1) * P, :], in_=o_st
                    )

    nc.compile()
    return nc


def _prep_inputs(x, cos, sin, Wq, Wk, Wv):
    """Host-side sharding/layout prep. Returns in_maps for 8 cores."""
    x = np.asarray(x, dtype=np.float32)
    cos = np.asarray(cos, dtype=np.float32)
    sin = np.asarray(sin, dtype=np.float32)

    def w_panels(w):
        # W.T [din, dout] -> [dc, p_din, k_din, c_dout] with d = k*128+p
        wt = np.ascontiguousarray(np.asarray(w, dtype=np.float32).T).astype(BF16)
        return np.ascontiguousarray(
            wt.reshape(KD, P, KD, P).transpose(2, 1, 0, 3)
        )

    wq_p = w_panels(Wq)
    wk_p = w_panels(Wk)
    # Wv.T [din, dout] -> [wb, p, k, 512] (dout blocks contiguous)
    wv_p = np.ascontiguousarray(
        np.asarray(Wv, dtype=np.float32)
        .T.astype(BF16)
        .reshape(KD, P, 4, FB)
        .transpose(2, 1, 0, 3)
    )

    # global row index of gathered slot s: pair rank h2 = s // NOWN owns the
    # rows with parity h2, so j_global(s) = 2*(s % NOWN) + h2
    slot = np.arange(S, dtype=np.int64)
    j_global = 2 * (slot % NOWN) + slot // NOWN

    in_maps = []
    for c in range(N_CORES):
        b, h = divmod(c, 2)
        rows = slice(h, None, 2)  # interleaved rows: h, h+2, h+4, ...
        xt = np.ascontiguousarray(
            x[b, rows, :].T.astype(BF16).reshape(KD, P, NOWN).transpose(1, 0, 2)
        )
        cos_t = np.ascontiguousarray(cos[rows].T.astype(BF16).reshape(KD, P, NOWN))
        sin_t = np.ascontiguousarray(sin[rows].T.astype(BF16).reshape(KD, P, NOWN))
        i_global = 2 * np.arange(NOWN, dtype=np.int64) + h
        # per jc, only the "mixed" i-quarter needs mask data
        mask_t = np.empty((NCJ, P, QW), dtype=BF16)
        for jc in range(NCJ):
            q = (jc % 8) // 2
            jg = j_global[jc * P : (jc + 1) * P][:, None]
            ig = i_global[q * QW : (q + 1) * QW][None, :]
            mask_t[jc] = (jg <= ig).astype(BF16)
        in_maps.append(
            {
                "x_t": xt,
                "wq": wq_p,
                "wk": wk_p,
                "wv": wv_p,
                "cos_t": cos_t,
                "sin_t": sin_t,
                "mask_t": mask_t,
            }
        )
    return in_maps


def _run(in_maps, trace=False, tmpdir=None):
    from concourse.bass_utils import run_bass_kernel_spmd

    if "nc" not in _CACHE:
        _CACHE["nc"] = _build()
    nc = _CACHE["nc"]
    return run_bass_kernel_spmd(
        nc, in_maps, list(range(N_CORES)), trace=trace, tmpdir=tmpdir
    )


def kernel(x, cos, sin, Wq, Wk, Wv):
    in_maps = _prep_inputs(x, cos, sin, Wq, Wk, Wv)
    res = _run(in_maps, trace=False)
    out = np.empty((B, S, D), dtype=np.float32)
    for c in range(N_CORES):
        b, h = divmod(c, 2)
        out[b, h::2, :] = res.results[c]["out"].T
    return out
